# revision 44
# baseline (speedup 1.0000x reference)
"""Trainium2 Bass kernel for the RNN-T JointNetwork problem.

Computes log_softmax(tanh(cat(enc, pred)) @ W.T + b) over the vocab dim
for logits of shape [B=4, T=200, U=50, V=1024].  Output is written in
bf16 (rel err ~5e-3 vs the fp32 reference, well inside the 2e-2 gate)
and upcast to fp32 on the host.

Strategy (data-parallel over the 800 flattened (b,t) rows, 100 per core):
  setup (per core, on device):
    teT  = tanh(encT_slice)          [512, 100]   (ACT)
    tpT  = tanh(predT_slice)         [512, 50]    (ACT)
    enc_p  = teT.T @ WeT             [100, 1024]  (PE, bf16 weights)
    pred_b = tpT.T @ WpT + b         [50, 1024]   (PE)
    lse    = ln(exp(enc_p) @ exp(pred_b).T)      [100, 50]
             -- exact: sum_v e^{a+b} = one tiny GEMM over exp'd factors,
             via PE transposes of enc_p/pred_b chunks (batched 4-per-PSUM
             tile so ACT exp evictions are few and large) + 8 matmuls,
             split into two t-halves so the first half lands early.
    -lse (bf16) is flattened into stationary row 54 of the one-hot
             operand by two small SBUF-to-SBUF DMAs (partition-major read
             of [t, 50] is flat row order), so the matmul itself applies
             the log-softmax shift.
  main loop over 40 row-tiles of 128 rows (row r = t*50 + u):
    mov[50:54] <- enc_p rows t0(k)..t0(k)+3      (SBUF-to-SBUF DMA, sync)
    x[r, v] = stat[k].T @ mov    ONE matmul per 512-col PSUM bank:
        stationary = 50 u-one-hot rows + 4 local-t one-hot rows + -lse
        values row (x ones row of mov), so pred_b[u] + enc_p[t] - lse[r]
        is a single accumulation pass at full bf16 PE rate.
    evictions are pure cast-copies to bf16 (ACT cols 0-511, DVE 512+),
    out tile -> DRAM on gpsimd/scalar queues alternating.

Input DMAs use (p c) row-interleaved layouts so each partition reads one
large contiguous DRAM block (the d-contraction order is a consistent
permutation of both W and the activations, so results are unchanged).
Measured on trn2: ~81 us HW exec (from 121.5 us baseline).
"""

import numpy as np

import concourse.bass as bass
import concourse.bacc as bacc
import concourse.tile as tile
from concourse import mybir
from concourse.bass_utils import run_bass_kernel_spmd

# Problem shapes (hardcoded per contract).
B, T, U, D, V = 4, 200, 50, 512, 1024
N_CORES = 8
BT = B * T                     # 800 flattened (b,t) rows
TPC = BT // N_CORES            # 100 (b,t) rows per core
ROWS = TPC * U                 # 5000 output rows per core
P = 128
PT = 128                       # rows per output tile (last tile ragged)
NT = (ROWS + PT - 1) // PT     # 40 row-tiles per core
NV = V // 512                  # PSUM bank = 512 fp32 cols per matmul
DC = D // P                    # 4 contraction chunks of 128 for D=512
VC = V // P                    # 8 vocab chunks of 128 (lse transposes)
NK = U + 5                     # stationary: 50 u-one-hot + 4 local-t + -lse
NMOV = 4                       # rotating moving-operand buffers

f32 = mybir.dt.float32
bf16 = mybir.dt.bfloat16

# t0c[k]: first enc_p row staged for tile k (clamped so t0c+4 <= TPC).
T0C = [min((k * PT) // U, TPC - 4) for k in range(NT)]

TRACE = False
LAST_RESULT = None

_CACHE = {}


def _patch_act_tables():
    """Pin Exp/Ln to the one table set containing both, so the activation
    table-load pass never alternates sets (Identity lives in every set)."""
    if getattr(bacc, "_joint_act_patch", False):
        return
    orig = bacc.get_activation_tables

    def patched(arch):
        t = dict(orig(arch))
        keep = "natural_log_exp_and_others"
        drop = {mybir.ActivationFunctionType.Exp, mybir.ActivationFunctionType.Ln}
        for name, fns in t.items():
            if name != keep:
                t[name] = set(fns) - drop
        return t

    bacc.get_activation_tables = patched
    bacc._joint_act_patch = True


def _build_stationary():
    """Per-row-tile combined one-hot stationary [NK, NT, P], shared by all
    cores.  Column c of tile k maps to row r = 128k + c:
      row u(r)             gets 1.0  (selects pred_b row)
      row 50 + t(r)-t0c[k] gets 1.0  (selects the staged enc_p row)
    Columns for rows >= ROWS are all-zero."""
    r = np.arange(NT * PT)
    valid = r < ROWS
    k_of = r // PT
    c_of = r % PT
    u = r % U
    t = np.minimum(r, ROWS - 1) // U
    off = t - np.asarray(T0C)[k_of]
    assert ((off[valid] >= 0) & (off[valid] <= 3)).all()
    stat = np.zeros((NK, NT, PT), dtype=np.float32)
    stat[u[valid], k_of[valid], c_of[valid]] = 1.0
    stat[(U + off)[valid], k_of[valid], c_of[valid]] = 1.0
    return stat


def _build_program():
    _patch_act_tables()
    nc = bacc.Bacc("TRN2", target_bir_lowering=False, debug=False,
                   num_devices=N_CORES)

    encT = nc.dram_tensor("encT", [D, TPC], f32, kind="ExternalInput")
    predT = nc.dram_tensor("predT", [D, U], f32, kind="ExternalInput")
    # W in bf16, split into enc/pred halves: halves the load and runs
    # matmuls at full PE rate; ~1e-3 relative rounding, inside tolerance.
    wTe = nc.dram_tensor("wTe", [D, V], bf16, kind="ExternalInput")
    wTp = nc.dram_tensor("wTp", [D, V], bf16, kind="ExternalInput")
    bias = nc.dram_tensor("bias", [1, V], bf16, kind="ExternalInput")
    stat = nc.dram_tensor("stat", [NK, NT, PT], bf16, kind="ExternalInput")
    ident = nc.dram_tensor("ident", [P, P], bf16, kind="ExternalInput")
    out = nc.dram_tensor("out", [ROWS, V], bf16, kind="ExternalOutput")

    ACT = mybir.ActivationFunctionType

    with tile.TileContext(nc) as tc:
        with (
            tc.tile_pool(name="consts", bufs=1) as consts,
            tc.tile_pool(name="ps", bufs=8, space=bass.MemorySpace.PSUM) as ps,
            tc.tile_pool(name="outs", bufs=10) as outs,
        ):
            # ---- input DMAs.  (p c) interleave: partition p reads rows
            #      [p*c .. p*c+c) as one contiguous DRAM block, so each W
            #      tensor is a single 128x8KB-packet DMA. ----
            encT_sb = consts.tile([P, DC, TPC], f32)
            encT_r = encT.ap().rearrange("(p c) t -> p c t", p=P)
            nc.scalar.dma_start(out=encT_sb[:, 0:2, :], in_=encT_r[:, 0:2, :])
            nc.scalar.dma_start(out=encT_sb[:, 2:4, :], in_=encT_r[:, 2:4, :])
            predT_sb = consts.tile([P, DC, U], f32)
            nc.scalar.dma_start(out=predT_sb[:], in_=predT.ap().rearrange(
                "(p c) u -> p c u", p=P))
            ident_sb = consts.tile([P, P], bf16)
            nc.sync.dma_start(out=ident_sb[:], in_=ident.ap())
            b_sb = consts.tile([1, V], bf16)
            nc.sync.dma_start(out=b_sb[:], in_=bias.ap())
            wt_e = consts.tile([P, DC, V], bf16)
            wTe_r = wTe.ap().rearrange("(p c) v -> p c v", p=P)
            nc.gpsimd.dma_start(out=wt_e[:, 0:1, :], in_=wTe_r[:, 0:1, :])
            nc.gpsimd.dma_start(out=wt_e[:, 1:2, :], in_=wTe_r[:, 1:2, :])
            nc.scalar.dma_start(out=wt_e[:, 2:4, :], in_=wTe_r[:, 2:4, :])
            wt_p = consts.tile([P, DC, V], bf16)
            wTp_r = wTp.ap().rearrange("(p c) v -> p c v", p=P)
            nc.sync.dma_start(out=wt_p[:, 0:2, :], in_=wTp_r[:, 0:2, :])
            nc.sync.dma_start(out=wt_p[:, 2:4, :], in_=wTp_r[:, 2:4, :])
            stat_sb = consts.tile([NK, NT, PT], bf16)
            nc.gpsimd.dma_start(out=stat_sb[:], in_=stat.ap())
            ones1 = consts.tile([1, U], bf16)
            nc.vector.memset(ones1[:], 1.0)

            # ---- tanh of activations (d on partitions), bf16 out ----
            teT = consts.tile([P, DC, TPC], bf16)
            nc.scalar.activation(teT[:, 0:2, :], encT_sb[:, 0:2, :], ACT.Tanh)
            nc.scalar.activation(teT[:, 2:4, :], encT_sb[:, 2:4, :], ACT.Tanh)
            tpT = consts.tile([P, DC, U], bf16)
            nc.scalar.activation(tpT[:], predT_sb[:], ACT.Tanh)

            # ---- enc_p[t, v] = sum_d teT[d, t] * We[v, d]  (2 halves) ----
            enc_p = consts.tile([TPC, V], bf16)
            for vc in range(NV):
                sl = slice(vc * 512, (vc + 1) * 512)
                eh = ps.tile([TPC, 512], f32, tag="ps", name=f"encps{vc}")
                for c in range(DC):
                    nc.tensor.matmul(eh[:], teT[:, c, :], wt_e[:, c, sl],
                                     start=(c == 0), stop=(c == DC - 1))
                nc.vector.tensor_copy(enc_p[:, sl], eh[:])

            # ---- pred_b[u, v] = sum_d tpT[d, u] * Wp[v, d] + b[v] ----
            movs = [consts.tile([NK, V], bf16, name=f"mov{i}")
                    for i in range(NMOV)]
            for vc in range(NV):
                sl = slice(vc * 512, (vc + 1) * 512)
                ph = ps.tile([U, 512], f32, tag="ps", name=f"predps{vc}")
                for c in range(DC):
                    nc.tensor.matmul(ph[:], tpT[:, c, :], wt_p[:, c, sl],
                                     start=(c == 0), stop=False)
                nc.tensor.matmul(ph[:], ones1[:], b_sb[:, sl],
                                 start=False, stop=True)
                nc.vector.tensor_copy(movs[0][0:U, sl], ph[:])
            for m in movs[1:]:
                nc.vector.tensor_copy(m[0:U, :], movs[0][0:U, :])
            ones_v = consts.tile([1, V], bf16)
            nc.vector.memset(ones_v[:], 1.0)
            for m in movs:
                nc.sync.dma_start(out=m[U + 4:U + 5, :], in_=ones_v[:])

            # ---- main-loop pieces (emitted out of line so the first
            #      couple of tiles' matmuls can precede the lse chain on
            #      the PE, hiding the lse DMA round-trip latency) ----
            xhalves = {}

            def mm_part(k):
                r0 = k * PT
                rows = min(PT, ROWS - r0)
                mov = movs[k % NMOV]
                nc.sync.dma_start(out=mov[U:U + 4, :],
                                  in_=enc_p[T0C[k]:T0C[k] + 4, :])
                x0 = ps.tile([PT, 512], f32, tag="ps", name="x0")
                nc.tensor.matmul(x0[:rows], stat_sb[:, k, :rows],
                                 mov[:, 0:512], start=True, stop=True)
                x1 = ps.tile([PT, 512], f32, tag="ps", name="x1")
                nc.tensor.matmul(x1[:rows], stat_sb[:, k, :rows],
                                 mov[:, 512:V], start=True, stop=True)
                xhalves[k] = (x0, x1, rows, r0)

            def evict_part(k):
                x0, x1, rows, r0 = xhalves.pop(k)
                o = outs.tile([PT, V], bf16, name="o")
                nc.scalar.copy(o[:rows, 0:512], x0[:rows])
                nc.vector.tensor_copy(o[:rows, 512:V], x1[:rows])
                eng = nc.gpsimd if k % 2 == 0 else nc.scalar
                eng.dma_start(out=out.ap()[r0:r0 + rows, :], in_=o[:rows])

            # ---- lse[t, u] = ln(sum_v exp(enc_p) * exp(pred_b)) ----
            Etr = consts.tile([P, VC, TPC], bf16)
            for g in range(2):
                pt = ps.tile([P, 4, TPC], bf16, tag="ps", name=f"ept{g}")
                for j in range(4):
                    c = 4 * g + j
                    nc.tensor.transpose(pt[:, j, :],
                                        enc_p[:, c * P:(c + 1) * P],
                                        ident_sb[:TPC, :TPC])
                nc.scalar.activation(Etr[:, 4 * g:4 * (g + 1), :], pt[:],
                                     ACT.Exp)
            Ptr = consts.tile([P, VC, U], bf16)
            for g in range(2):
                pt = ps.tile([P, 4, U], bf16, tag="ps", name=f"ppt{g}")
                for j in range(4):
                    c = 4 * g + j
                    nc.tensor.transpose(pt[:, j, :],
                                        movs[0][0:U, c * P:(c + 1) * P],
                                        ident_sb[:U, :U])
                nc.scalar.activation(Ptr[:, 4 * g:4 * (g + 1), :], pt[:],
                                     ACT.Exp)
            # -lse lands as stationary row 54 (one partition, contiguous),
            # so the matmul itself applies the log-softmax shift and the
            # evictions below are pure cast-copies.  The S -> Ln -> neg ->
            # flatten chain runs in two t-halves split at t=64 (= exactly
            # 25 row-tiles of 128), so the stationary row for tiles 0-24
            # lands ~3us earlier and the first matmuls aren't gated on the
            # full chain.  Flatten = two SBUF-to-SBUF DMAs (partition-major
            # read of [t, 50] is flat row order); pad columns stay zero.
            nflat = consts.tile([1, NT * PT], bf16)
            nc.vector.memset(nflat[:, ROWS:], 0.0)
            for (t0, t1, k0, k1) in ((0, 64, 0, 25), (64, TPC, 25, NT)):
                Sp = ps.tile([t1 - t0, U], f32, tag="ps", name=f"S{t0}")
                for c in range(VC):
                    nc.tensor.matmul(Sp[:], Etr[:, c, t0:t1], Ptr[:, c, :],
                                     start=(c == 0), stop=(c == VC - 1))
                lse_p = consts.tile([t1 - t0, U], f32, name=f"lse{t0}")
                nc.scalar.activation(lse_p[:], Sp[:], ACT.Ln)
                nlse_p = consts.tile([t1 - t0, U], bf16, name=f"nlse{t0}")
                nc.vector.tensor_scalar_mul(nlse_p[:], lse_p[:], -1.0)
                if (t1 - t0) * U == (k1 - k0) * PT:
                    # exact fit: one direct DMA, nothing else gates mm(k0..)
                    nc.sync.dma_start(out=stat_sb[U + 4:U + 5, k0:k1, :],
                                      in_=nlse_p[:])
                else:
                    nc.sync.dma_start(out=nflat[:, t0 * U:t1 * U],
                                      in_=nlse_p[:])
                    nc.sync.dma_start(out=stat_sb[U + 4:U + 5, k0:k1, :],
                                      in_=nflat[:, k0 * PT:k1 * PT])

            # ---- main loop over row tiles.  Matmuls lead evictions by
            #      three tiles in program order (4 tiles x 2 PSUM bufs in
            #      flight = exactly the 8 banks); the sync queue carries
            #      only the tiny mov DMAs, outputs ride gpsimd/scalar.
            LAG = 2
            for k in range(LAG):
                mm_part(k)
            for k in range(LAG, NT):
                mm_part(k)
                evict_part(k - LAG)
            for k in range(NT - LAG, NT):
                evict_part(k)

    nc.compile()
    return nc


def kernel(enc_out, pred_out, W, b):
    global LAST_RESULT
    enc_out = np.asarray(enc_out, dtype=np.float32)
    pred_out = np.asarray(pred_out, dtype=np.float32)
    W = np.asarray(W, dtype=np.float32)
    b = np.asarray(b, dtype=np.float32)

    if "nc" not in _CACHE:
        _CACHE["nc"] = _build_program()
        _CACHE["stat"] = _build_stationary()
    nc = _CACHE["nc"]

    import ml_dtypes
    wT = np.ascontiguousarray(W.T).astype(ml_dtypes.bfloat16)   # [2D, V]
    wTe = np.ascontiguousarray(wT[:D])
    wTp = np.ascontiguousarray(wT[D:])
    stat = _CACHE["stat"].astype(ml_dtypes.bfloat16)
    ident = np.eye(P, dtype=ml_dtypes.bfloat16)
    bias = np.ascontiguousarray(b.reshape(1, V)).astype(ml_dtypes.bfloat16)
    enc_flat = enc_out.reshape(BT, D)                 # [800, 512]

    in_maps = []
    for c in range(N_CORES):
        bt0 = c * TPC
        b_idx = bt0 // T
        in_maps.append({
            "encT": np.ascontiguousarray(enc_flat[bt0:bt0 + TPC].T),
            "predT": np.ascontiguousarray(pred_out[b_idx].T),
            "wTe": wTe,
            "wTp": wTp,
            "bias": bias,
            "stat": stat,
            "ident": ident,
        })

    res = run_bass_kernel_spmd(nc, in_maps, core_ids=list(range(N_CORES)),
                               trace=TRACE)
    LAST_RESULT = res
    full = np.concatenate([np.asarray(r["out"]).astype(np.float32)
                           for r in res.results], axis=0)
    return full.reshape(B, T, U, V)


# revision 45
# speedup vs baseline: 1.1330x; 1.1330x over previous
"""Trainium2 Bass kernel for the RNN-T JointNetwork problem.

Computes log_softmax(tanh(cat(enc, pred)) @ W.T + b) over the vocab dim
for logits of shape [B=4, T=200, U=50, V=1024].  Output is written in
bf16 (rel err ~5e-3 vs the fp32 reference, well inside the 2e-2 gate)
and upcast to fp32 on the host.

Strategy (data-parallel over the 800 flattened (b,t) rows, 100 per core):
  setup (per core, on device):
    teT  = tanh(encT_slice)          [512, 100]   (ACT)
    tpT  = tanh(predT_slice)         [512, 50]    (ACT)
    enc_p  = teT.T @ WeT             [100, 1024]  (PE, bf16 weights)
    pred_b = tpT.T @ WpT + b         [50, 1024]   (PE)
    lse    = ln(exp(enc_p) @ exp(pred_b).T)      [100, 50]
             -- exact: sum_v e^{a+b} = one tiny GEMM over exp'd factors,
             via PE transposes of enc_p/pred_b chunks (batched 4-per-PSUM
             tile so ACT exp evictions are few and large) + 8 matmuls,
             split into two t-halves so the first half lands early.
    -lse (bf16) is flattened into stationary row 54 of the one-hot
             operand by two small SBUF-to-SBUF DMAs (partition-major read
             of [t, 50] is flat row order), so the matmul itself applies
             the log-softmax shift.
  main loop over 40 row-tiles of 128 rows (row r = t*50 + u):
    mov[50:54] <- enc_p rows t0(k)..t0(k)+3      (SBUF-to-SBUF DMA, sync)
    x[r, v] = stat[k].T @ mov    ONE matmul per 512-col PSUM bank:
        stationary = 50 u-one-hot rows + 4 local-t one-hot rows + -lse
        values row (x ones row of mov), so pred_b[u] + enc_p[t] - lse[r]
        is a single accumulation pass at full bf16 PE rate.
    evictions are pure cast-copies to bf16 (ACT cols 0-511, DVE 512+),
    out tile -> DRAM on gpsimd/scalar queues alternating.

Input DMAs use (p c) row-interleaved layouts so each partition reads one
large contiguous DRAM block (the d-contraction order is a consistent
permutation of both W and the activations, so results are unchanged).
Measured on trn2: ~81 us HW exec (from 121.5 us baseline).
"""

import numpy as np

import concourse.bass as bass
import concourse.bacc as bacc
import concourse.tile as tile
from concourse import mybir
from concourse.bass_utils import run_bass_kernel_spmd

# Problem shapes (hardcoded per contract).
B, T, U, D, V = 4, 200, 50, 512, 1024
N_CORES = 8
BT = B * T                     # 800 flattened (b,t) rows
TPC = BT // N_CORES            # 100 (b,t) rows per core
ROWS = TPC * U                 # 5000 output rows per core
P = 128
PT = 128                       # rows per output tile (last tile ragged)
NT = (ROWS + PT - 1) // PT     # 40 row-tiles per core
NV = V // 512                  # PSUM bank = 512 fp32 cols per matmul
DC = D // P                    # 4 contraction chunks of 128 for D=512
VC = V // P                    # 8 vocab chunks of 128 (lse transposes)
NK = U + 5                     # stationary: 50 u-one-hot + 4 local-t + -lse
NMOV = 4                       # rotating moving-operand buffers

f32 = mybir.dt.float32
bf16 = mybir.dt.bfloat16

# t0c[k]: first enc_p row staged for tile k (clamped so t0c+4 <= TPC).
T0C = [min((k * PT) // U, TPC - 4) for k in range(NT)]

TRACE = False
LAST_RESULT = None

_CACHE = {}


def _patch_act_tables():
    """Pin Exp/Ln to the one table set containing both, so the activation
    table-load pass never alternates sets (Identity lives in every set)."""
    if getattr(bacc, "_joint_act_patch", False):
        return
    orig = bacc.get_activation_tables

    def patched(arch):
        t = dict(orig(arch))
        keep = "natural_log_exp_and_others"
        drop = {mybir.ActivationFunctionType.Exp, mybir.ActivationFunctionType.Ln}
        for name, fns in t.items():
            if name != keep:
                t[name] = set(fns) - drop
        return t

    bacc.get_activation_tables = patched
    bacc._joint_act_patch = True


def _build_stationary():
    """Per-row-tile combined one-hot stationary [NK, NT, P], shared by all
    cores.  Column c of tile k maps to row r = 128k + c:
      row u(r)             gets 1.0  (selects pred_b row)
      row 50 + t(r)-t0c[k] gets 1.0  (selects the staged enc_p row)
    Columns for rows >= ROWS are all-zero."""
    r = np.arange(NT * PT)
    valid = r < ROWS
    k_of = r // PT
    c_of = r % PT
    u = r % U
    t = np.minimum(r, ROWS - 1) // U
    off = t - np.asarray(T0C)[k_of]
    assert ((off[valid] >= 0) & (off[valid] <= 3)).all()
    stat = np.zeros((NK, NT, PT), dtype=np.float32)
    stat[u[valid], k_of[valid], c_of[valid]] = 1.0
    stat[(U + off)[valid], k_of[valid], c_of[valid]] = 1.0
    return stat


def _build_program():
    _patch_act_tables()
    nc = bacc.Bacc("TRN2", target_bir_lowering=False, debug=False,
                   num_devices=N_CORES)

    encT = nc.dram_tensor("encT", [D, TPC], f32, kind="ExternalInput")
    predT = nc.dram_tensor("predT", [D, U], f32, kind="ExternalInput")
    # W in bf16, split into enc/pred halves: halves the load and runs
    # matmuls at full PE rate; ~1e-3 relative rounding, inside tolerance.
    wTe = nc.dram_tensor("wTe", [D, V], bf16, kind="ExternalInput")
    wTp = nc.dram_tensor("wTp", [D, V], bf16, kind="ExternalInput")
    bias = nc.dram_tensor("bias", [1, V], bf16, kind="ExternalInput")
    stat = nc.dram_tensor("stat", [NK, NT, PT], bf16, kind="ExternalInput")
    ident = nc.dram_tensor("ident", [P, P], bf16, kind="ExternalInput")
    out = nc.dram_tensor("out", [ROWS, V], bf16, kind="ExternalOutput")

    ACT = mybir.ActivationFunctionType

    with tile.TileContext(nc) as tc:
        with (
            tc.tile_pool(name="consts", bufs=1) as consts,
            tc.tile_pool(name="ps", bufs=8, space=bass.MemorySpace.PSUM) as ps,
            tc.tile_pool(name="outs", bufs=10) as outs,
        ):
            # ---- input DMAs.  (p c) interleave: partition p reads rows
            #      [p*c .. p*c+c) as one contiguous DRAM block, so each W
            #      tensor is a single 128x8KB-packet DMA. ----
            encT_sb = consts.tile([P, DC, TPC], f32)
            encT_r = encT.ap().rearrange("(p c) t -> p c t", p=P)
            nc.scalar.dma_start(out=encT_sb[:, 0:2, :], in_=encT_r[:, 0:2, :])
            nc.scalar.dma_start(out=encT_sb[:, 2:4, :], in_=encT_r[:, 2:4, :])
            predT_sb = consts.tile([P, DC, U], f32)
            nc.scalar.dma_start(out=predT_sb[:], in_=predT.ap().rearrange(
                "(p c) u -> p c u", p=P))
            ident_sb = consts.tile([P, P], bf16)
            nc.sync.dma_start(out=ident_sb[:], in_=ident.ap())
            b_sb = consts.tile([1, V], bf16)
            nc.sync.dma_start(out=b_sb[:], in_=bias.ap())
            wt_e = consts.tile([P, DC, V], bf16)
            wTe_r = wTe.ap().rearrange("(p c) v -> p c v", p=P)
            nc.gpsimd.dma_start(out=wt_e[:, 0:1, :], in_=wTe_r[:, 0:1, :])
            nc.gpsimd.dma_start(out=wt_e[:, 1:2, :], in_=wTe_r[:, 1:2, :])
            nc.scalar.dma_start(out=wt_e[:, 2:4, :], in_=wTe_r[:, 2:4, :])
            wt_p = consts.tile([P, DC, V], bf16)
            wTp_r = wTp.ap().rearrange("(p c) v -> p c v", p=P)
            nc.sync.dma_start(out=wt_p[:, 0:2, :], in_=wTp_r[:, 0:2, :])
            nc.sync.dma_start(out=wt_p[:, 2:4, :], in_=wTp_r[:, 2:4, :])
            stat_sb = consts.tile([NK, NT, PT], bf16)
            nc.gpsimd.dma_start(out=stat_sb[:], in_=stat.ap())
            ones1 = consts.tile([1, U], bf16)
            nc.vector.memset(ones1[:], 1.0)

            # ---- tanh of activations (d on partitions), bf16 out ----
            teT = consts.tile([P, DC, TPC], bf16)
            nc.scalar.activation(teT[:, 0:2, :], encT_sb[:, 0:2, :], ACT.Tanh)
            nc.scalar.activation(teT[:, 2:4, :], encT_sb[:, 2:4, :], ACT.Tanh)
            tpT = consts.tile([P, DC, U], bf16)
            nc.scalar.activation(tpT[:], predT_sb[:], ACT.Tanh)

            # ---- enc_p[t, v] = sum_d teT[d, t] * We[v, d]  (2 halves) ----
            enc_p = consts.tile([TPC, V], bf16)
            for vc in range(NV):
                sl = slice(vc * 512, (vc + 1) * 512)
                eh = ps.tile([TPC, 512], f32, tag="ps", name=f"encps{vc}")
                for c in range(DC):
                    nc.tensor.matmul(eh[:], teT[:, c, :], wt_e[:, c, sl],
                                     start=(c == 0), stop=(c == DC - 1))
                nc.vector.tensor_copy(enc_p[:, sl], eh[:])

            # ---- pred_b[u, v] = sum_d tpT[d, u] * Wp[v, d] + b[v] ----
            movs = [consts.tile([NK, V], bf16, name=f"mov{i}")
                    for i in range(NMOV)]
            for vc in range(NV):
                sl = slice(vc * 512, (vc + 1) * 512)
                ph = ps.tile([U, 512], f32, tag="ps", name=f"predps{vc}")
                for c in range(DC):
                    nc.tensor.matmul(ph[:], tpT[:, c, :], wt_p[:, c, sl],
                                     start=(c == 0), stop=False)
                nc.tensor.matmul(ph[:], ones1[:], b_sb[:, sl],
                                 start=False, stop=True)
                nc.vector.tensor_copy(movs[0][0:U, sl], ph[:])
            for m in movs[1:]:
                nc.vector.tensor_copy(m[0:U, :], movs[0][0:U, :])
            ones_v = consts.tile([1, V], bf16)
            nc.vector.memset(ones_v[:], 1.0)
            for m in movs:
                nc.sync.dma_start(out=m[U + 4:U + 5, :], in_=ones_v[:])

            # ---- main-loop pieces (emitted out of line so the first
            #      couple of tiles' matmuls can precede the lse chain on
            #      the PE, hiding the lse DMA round-trip latency) ----
            xhalves = {}

            def mm_part(k):
                r0 = k * PT
                rows = min(PT, ROWS - r0)
                mov = movs[k % NMOV]
                nc.sync.dma_start(out=mov[U:U + 4, :],
                                  in_=enc_p[T0C[k]:T0C[k] + 4, :])
                x0 = ps.tile([PT, 512], f32, tag="ps", name="x0")
                nc.tensor.matmul(x0[:rows], stat_sb[:, k, :rows],
                                 mov[:, 0:512], start=True, stop=True)
                x1 = ps.tile([PT, 512], f32, tag="ps", name="x1")
                nc.tensor.matmul(x1[:rows], stat_sb[:, k, :rows],
                                 mov[:, 512:V], start=True, stop=True)
                xhalves[k] = (x0, x1, rows, r0)

            def evict_part(k):
                x0, x1, rows, r0 = xhalves.pop(k)
                o = outs.tile([PT, V], bf16, name="o")
                nc.scalar.copy(o[:rows, 0:512], x0[:rows])
                nc.vector.tensor_copy(o[:rows, 512:V], x1[:rows])
                eng = nc.gpsimd if k % 2 == 0 else nc.scalar
                eng.dma_start(out=out.ap()[r0:r0 + rows, :], in_=o[:rows])

            # ---- lse[t, u] = ln(sum_v exp(enc_p) * exp(pred_b)) ----
            Etr = consts.tile([P, VC, TPC], bf16)
            for g in range(2):
                pt = ps.tile([P, 4, TPC], bf16, tag="ps", name=f"ept{g}")
                for j in range(4):
                    c = 4 * g + j
                    nc.tensor.transpose(pt[:, j, :],
                                        enc_p[:, c * P:(c + 1) * P],
                                        ident_sb[:TPC, :TPC])
                nc.scalar.activation(Etr[:, 4 * g:4 * (g + 1), :], pt[:],
                                     ACT.Exp)
            Ptr = consts.tile([P, VC, U], bf16)
            for g in range(2):
                pt = ps.tile([P, 4, U], bf16, tag="ps", name=f"ppt{g}")
                for j in range(4):
                    c = 4 * g + j
                    nc.tensor.transpose(pt[:, j, :],
                                        movs[0][0:U, c * P:(c + 1) * P],
                                        ident_sb[:U, :U])
                nc.scalar.activation(Ptr[:, 4 * g:4 * (g + 1), :], pt[:],
                                     ACT.Exp)
            # -lse lands as stationary row 54 (one partition, contiguous),
            # so the matmul itself applies the log-softmax shift and the
            # evictions below are pure cast-copies.  The S -> Ln -> neg ->
            # flatten chain runs in two t-halves split at t=64 (= exactly
            # 25 row-tiles of 128), so the stationary row for tiles 0-24
            # lands ~3us earlier and the first matmuls aren't gated on the
            # full chain.  Flatten = two SBUF-to-SBUF DMAs (partition-major
            # read of [t, 50] is flat row order); pad columns stay zero.
            nflat = consts.tile([1, NT * PT], bf16)
            nc.vector.memset(nflat[:, ROWS:], 0.0)
            for (t0, t1, k0, k1) in ((0, 64, 0, 25), (64, TPC, 25, NT)):
                Sp = ps.tile([t1 - t0, U], f32, tag="ps", name=f"S{t0}")
                for c in range(VC):
                    nc.tensor.matmul(Sp[:], Etr[:, c, t0:t1], Ptr[:, c, :],
                                     start=(c == 0), stop=(c == VC - 1))
                lse_p = consts.tile([t1 - t0, U], f32, name=f"lse{t0}")
                nc.scalar.activation(lse_p[:], Sp[:], ACT.Ln)
                nlse_p = consts.tile([t1 - t0, U], bf16, name=f"nlse{t0}")
                nc.vector.tensor_scalar_mul(nlse_p[:], lse_p[:], -1.0)
                if (t1 - t0) * U == (k1 - k0) * PT:
                    # exact fit: one direct DMA, nothing else gates mm(k0..)
                    nc.sync.dma_start(out=stat_sb[U + 4:U + 5, k0:k1, :],
                                      in_=nlse_p[:])
                else:
                    nc.sync.dma_start(out=nflat[:, t0 * U:t1 * U],
                                      in_=nlse_p[:])
                    nc.sync.dma_start(out=stat_sb[U + 4:U + 5, k0:k1, :],
                                      in_=nflat[:, k0 * PT:k1 * PT])

            # ---- main loop over row tiles.  Matmuls lead evictions by
            #      three tiles in program order (4 tiles x 2 PSUM bufs in
            #      flight = exactly the 8 banks); the sync queue carries
            #      only the tiny mov DMAs, outputs ride gpsimd/scalar.
            LAG = 3
            for k in range(LAG):
                mm_part(k)
            for k in range(LAG, NT):
                mm_part(k)
                evict_part(k - LAG)
            for k in range(NT - LAG, NT):
                evict_part(k)

    nc.compile()
    return nc


def kernel(enc_out, pred_out, W, b):
    global LAST_RESULT
    enc_out = np.asarray(enc_out, dtype=np.float32)
    pred_out = np.asarray(pred_out, dtype=np.float32)
    W = np.asarray(W, dtype=np.float32)
    b = np.asarray(b, dtype=np.float32)

    if "nc" not in _CACHE:
        _CACHE["nc"] = _build_program()
        _CACHE["stat"] = _build_stationary()
    nc = _CACHE["nc"]

    import ml_dtypes
    wT = np.ascontiguousarray(W.T).astype(ml_dtypes.bfloat16)   # [2D, V]
    wTe = np.ascontiguousarray(wT[:D])
    wTp = np.ascontiguousarray(wT[D:])
    stat = _CACHE["stat"].astype(ml_dtypes.bfloat16)
    ident = np.eye(P, dtype=ml_dtypes.bfloat16)
    bias = np.ascontiguousarray(b.reshape(1, V)).astype(ml_dtypes.bfloat16)
    enc_flat = enc_out.reshape(BT, D)                 # [800, 512]

    in_maps = []
    for c in range(N_CORES):
        bt0 = c * TPC
        b_idx = bt0 // T
        in_maps.append({
            "encT": np.ascontiguousarray(enc_flat[bt0:bt0 + TPC].T),
            "predT": np.ascontiguousarray(pred_out[b_idx].T),
            "wTe": wTe,
            "wTp": wTp,
            "bias": bias,
            "stat": stat,
            "ident": ident,
        })

    res = run_bass_kernel_spmd(nc, in_maps, core_ids=list(range(N_CORES)),
                               trace=TRACE)
    LAST_RESULT = res
    full = np.concatenate([np.asarray(r["out"]).astype(np.float32)
                           for r in res.results], axis=0)
    return full.reshape(B, T, U, V)
